# revision 31
# baseline (speedup 1.0000x reference)
"""Trainium2 Bass kernel for cross-attention (b=4, nq=2048, nkv=1024,
qdim=1024, cdim=768, heads=16, dim_head=64).

Sharding: 8 cores = batch(4) x nq-half(2). Each core computes a disjoint
[1024, 1024] slice of the output; no collectives needed.

Host-side prep (part of kernel()):
  - KV compaction: the per-key mask drops ~half the keys. Gather the
    unmasked context rows per batch, pad with zeros to a multiple of 128
    (NKVP = nkc*128 kv positions). softmax over masked entries ==
    softmax over the unmasked subset, so this is exact.
  - x and ctx are uploaded pre-transposed (xt = x^T, ctx_t = ctx^T), which
    removes all PE transpose work on device.
  - mask64 ([128, nkc*64], bf16): per-kv-chunk validity column-broadcast
    to 64 cols; its first 8 cols per chunk seed the validity columns of
    the augmented V layout. Pad rows of ctx are zero so V pad rows are 0.
  - bob ([128, 1024], bf16): output bias row-replicated so the final
    PSUM->SBUF move is a DVE add (no bias matmuls).

Per-core device algorithm (PE cost on TRN2 is streamed-rows only:
time = out_free_size x 0.4167ns per matmul, so minimize total rows):
  KT = Wk^T @ CT   [inner, NKVP]
  V' = CT^T @ Wv   [NKVP, aug]  augmented layout: per head 65 cols
                   (64 V cols + 1 validity col; validity excludes pad
                   rows from the softmax denominator)
  QT = Wq^T @ XT   [inner, nq]  (p=0 upfront; p>=1 interleaved into
                   the attention stream as PE filler)
  per (hf, head-pair p), per kv-chunk c:
    S^T = K_h @ Q_h^T            (row-tiled K=64: 2 heads concurrent)
    ES  = exp(SCALE * S^T)       (ScalarE, PSUM->SBUF, bf16)
    po0[0:65]   += [V_h0 | 1]^T @ ES_h0   (rows 0:64 = O^T, row 64 = rowsum)
    po1[63:128] += [1 | V_h1]^T @ ES_h1   (row 63 = rowsum, rows 64:128 = O^T)
  epilogue: rcp = 1/rowsum (two [1,512] DVE recips); rtb = ones^T @ rcp
  (two K=1 PE matmuls broadcast rcp across partitions); rtb -> SBUF;
  OT = po * rtb (two [64,512] DVE muls)
  out = OT^T @ Wo + bob  (bias folded into the PSUM->SBUF DVE add);
  out-proj for the first query half is interleaved into the hf=1
  attention stream, the rest trails the last pair.

Scores are O(1) by construction so unshifted exp is safe.
"""

import numpy as np
from contextlib import ExitStack

import ml_dtypes

import concourse.bass as bass
import concourse.mybir as mybir
import concourse.tile as tile
from concourse import bacc
from concourse.bass_utils import run_bass_kernel_spmd

F32 = mybir.dt.float32
BF16 = mybir.dt.bfloat16
AF = mybir.ActivationFunctionType
NPBF16 = ml_dtypes.bfloat16

NQ = 1024      # queries per core
NKV = 1024
QD = 1024
CD = 768
H = 16
D = 64
INNER = 1024
SCALE = D ** -0.5
P = 128
NQC = NQ // P      # 8 nq chunks
QDC = QD // P      # 8
CDC = CD // P      # 6
HP = H // 2        # 8 head pairs
VW = D + 1         # augmented V width per head (64 V cols + validity)
CW = H * VW        # augmented V cols per kv chunk


def _emit(tc, io, nkc, stages=("proj", "attn", "out")):
    nc = tc.nc
    xt_d, ctx_t, mask64_d, wq_d, wk_d, wv_d, wo_d, bob_d, out_d = io
    NKVP = nkc * P

    with ExitStack() as top:
        const = top.enter_context(tc.tile_pool(name="const", bufs=1))
        big = top.enter_context(tc.tile_pool(name="big", bufs=1))

        ones = const.tile([P, P], BF16, tag="ones")
        nc.vector.memset(ones[:], 1.0)

        ot = big.tile([P, QDC * NQ], BF16, tag="ot")     # O^T: chunk k cols k*NQ..
        qt = big.tile([P, HP * NQ], BF16, tag="qt")      # Q^T: chunk p cols p*NQ..
        kt = big.tile([P, HP * NKVP], BF16, tag="kt")
        vt = big.tile([P, nkc * CW], BF16, tag="vt")     # augmented V

        # ---- input DMAs in need-order (one DMA resource serializes all
        # queues, so issue order ~= arrival order): K deps, V deps, Q deps,
        # out deps ----
        # ct/wk arrive in two halves: the first K-proj matmuls start ~4us
        # earlier than a monolithic load allows, with only one p-state ramp
        ct = big.tile([P, CDC * NKVP], BF16, tag="ct")
        ct3 = ct.rearrange("p (k n) -> p k n", n=NKVP)
        wkt = big.tile([P, CDC * INNER], BF16, tag="wk")
        wk3 = wkt.rearrange("p (k n) -> p k n", n=INNER)
        hc = CDC // 2
        for h in range(2):
            k0, k1 = h * hc, (h + 1) * hc
            nc.sync.dma_start(out=ct3[:, k0:k1, :],
                              in_=ctx_t[k0 * P:k1 * P, :].rearrange(
                                  "(k p) n -> p k n", p=P))
            nc.scalar.dma_start(out=wk3[:, k0:k1, :],
                                in_=wk_d[k0 * P:k1 * P, :].rearrange(
                                    "(k p) n -> p k n", p=P))
        wk = [wk3[:, k, :] for k in range(CDC)]
        wvt = big.tile([P, CDC * INNER], BF16, tag="wv")
        wv3 = wvt.rearrange("p (k n) -> p k n", n=INNER)
        nc.sync.dma_start(out=wv3[:], in_=wv_d.rearrange("(k p) n -> p k n", p=P))
        wv = [wv3[:, k, :] for k in range(CDC)]
        mask64 = const.tile([P, nkc * D], BF16, tag="mask64")
        nc.scalar.dma_start(out=mask64[:], in_=mask64_d)
        xt = big.tile([P, QDC * NQ], BF16, tag="xt")
        xt3 = xt.rearrange("p (k n) -> p k n", n=NQ)
        nc.sync.dma_start(out=xt3[:], in_=xt_d.rearrange("(k p) n -> p k n", p=P))
        wqt = big.tile([P, QDC * INNER], BF16, tag="wq")
        wq3 = wqt.rearrange("p (k n) -> p k n", n=INNER)
        nc.scalar.dma_start(out=wq3[:], in_=wq_d.rearrange("(k p) n -> p k n", p=P))
        wq = [wq3[:, k, :] for k in range(QDC)]
        wo = big.tile([P, QDC * QD], BF16, tag="wo")
        wo3 = wo.rearrange("p (k n) -> p k n", n=QD)
        nc.sync.dma_start(out=wo3[:], in_=wo_d.rearrange("(k p) n -> p k n", p=P))
        bob = const.tile([P, QD], BF16, tag="bob")
        nc.scalar.dma_start(out=bob[:], in_=bob_d)

        # ---- K projection: KT[inner, kv] (psum must not cross 2KB banks).
        # k-outer over two 4-pair phases so each (ct, wk) chunk is consumed
        # as soon as its DMA lands ----
        ksplits = [(lo, min(lo + 512, NKVP)) for lo in range(0, NKVP, 512)]
        proj_ctx = ExitStack()
        kp_ctx = ExitStack()
        pj_ps = kp_ctx.enter_context(tc.tile_pool(name="pj_ps", bufs=1, space="PSUM"))
        # phase 1 (p=0..3): k-outer, paced by the chunked DMAs
        tiles = [pj_ps.tile([P, NKVP], F32, tag=f"pj{i}", name=f"pjk0_{i}")
                 for i in range(4)]
        for k in range(CDC):
            for i, ps in enumerate(tiles):
                for (lo, hi) in ksplits:
                    nc.tensor.matmul(
                        ps[:, lo:hi], wk[k][:, i * P:(i + 1) * P],
                        ct[:, k * NKVP + lo: k * NKVP + hi],
                        start=(k == 0), stop=(k == CDC - 1),
                        skip_group_check=True)
        for i, ps in enumerate(tiles):
            nc.vector.tensor_copy(kt[:, i * NKVP:(i + 1) * NKVP], ps[:])
        # phase 2 (p=4..7): p-outer with per-p copy so banks free
        # progressively for the V projection
        for p in range(4, HP):
            ps = pj_ps.tile([P, NKVP], F32, tag=f"pj{p - 4}", name=f"pjk1_{p}")
            for k in range(CDC):
                for (lo, hi) in ksplits:
                    nc.tensor.matmul(
                        ps[:, lo:hi], wk[k][:, p * P:(p + 1) * P],
                        ct[:, k * NKVP + lo: k * NKVP + hi],
                        start=(k == 0), stop=(k == CDC - 1),
                        skip_group_check=True)
            nc.vector.tensor_copy(kt[:, p * NKVP:(p + 1) * NKVP], ps[:])
        kp_ctx.close()

        # ---- V projection into augmented layout ----
        vt5 = vt.rearrange("p (c hp two m) -> p c hp two m",
                           c=nkc, hp=HP, two=2, m=VW)
        vq_ps = proj_ctx.enter_context(tc.tile_pool(name="vq_ps", bufs=4, space="PSUM"))
        for c in range(nkc):
            for hf2 in range(2):
                ps = vq_ps.tile([P, 512], F32, tag="vq", name=f"pjv{c}_{hf2}")
                for k in range(CDC):
                    nc.tensor.matmul(
                        ps[:], ct[:, k * NKVP + c * P: k * NKVP + (c + 1) * P],
                        wv[k][:, hf2 * 512:(hf2 + 1) * 512],
                        start=(k == 0), stop=(k == CDC - 1))
                ps4 = ps.rearrange("p (f two d) -> p f two d", f=4, two=2, d=D)
                nc.vector.tensor_copy(
                    vt5[:, c, hf2 * 4:(hf2 + 1) * 4, :, 0:D], ps4[:])
            mval = mask64[:, c * D: c * D + H].rearrange(
                "p (a t o) -> p a t o", t=2, o=1)
            nc.vector.tensor_copy(vt5[:, c, :, :, D:VW], mval[:])

        # ---- Q projection as op-lists (p=0 upfront, p>=1 as attention
        # filler) ----
        def qproj_ops(p, pool, tag):
            ops, st = [], {}
            for hf2 in range(2):
                def mk(k, hf2=hf2):
                    def go():
                        if k == 0:
                            st[hf2] = pool.tile([P, 512], F32, tag=tag,
                                                name=f"pjq{p}_{hf2}")
                        nc.tensor.matmul(
                            st[hf2][:], wq[k][:, p * P:(p + 1) * P],
                            xt[:, k * NQ + hf2 * 512: k * NQ + (hf2 + 1) * 512],
                            start=(k == 0), stop=(k == QDC - 1),
                            skip_group_check=True)
                    return go
                for k in range(QDC):
                    ops.append(("pe", mk(k)))
                def cp(hf2=hf2):
                    nc.vector.tensor_copy(
                        qt[:, p * NQ + hf2 * 512: p * NQ + (hf2 + 1) * 512],
                        st[hf2][:])
                ops.append(("dve", cp))
            return ops

        for _, f in qproj_ops(0, vq_ps, "vq"):
            f()

        # ---- out projection as op-lists (m=0..3 as hf=1 attention filler,
        # m=4..7 trailing) ----
        def outproj_ops(m, pool, tag, sb_pool):
            ops, st = [], {}
            for n in range(2):
                def mm(k, n=n):
                    def go():
                        if k == 0 and n == 0:
                            st["sb"] = sb_pool.tile([P, QD], BF16, tag="osb",
                                                    name=f"osb{m}")
                        if k == 0:
                            st[n] = pool.tile([P, 512], F32, tag=tag,
                                              name=f"ops{m}_{n}")
                        nc.tensor.matmul(
                            st[n][:],
                            ot[:, k * NQ + m * P: k * NQ + (m + 1) * P],
                            wo3[:, k, n * 512:(n + 1) * 512],
                            start=(k == 0), stop=(k == QDC - 1),
                            skip_group_check=True)
                    return go
                for k in range(QDC):
                    ops.append(("pe", mm(k)))
                def cp(n=n):
                    nc.vector.tensor_add(
                        st["sb"][:, n * 512:(n + 1) * 512], st[n][:],
                        bob[:, n * 512:(n + 1) * 512])
                ops.append(("dve", cp))
            def dma():
                nc.sync.dma_start(out=out_d[m * P:(m + 1) * P, :], in_=st["sb"][:])
            ops.append(("dma", dma))
            return ops

        # ---- attention, hf-outer; Qproj/outproj interleaved as PE filler ----
        proj_ctx.close()
        out_sb = top.enter_context(tc.tile_pool(name="out_sb", bufs=3))
        attn_ctx = ExitStack()
        es_pool = attn_ctx.enter_context(tc.tile_pool(name="esp", bufs=3))
        rcp_pool = attn_ctx.enter_context(tc.tile_pool(name="rcpp", bufs=2))
        ps_s = attn_ctx.enter_context(tc.tile_pool(name="ps_s", bufs=2, space="PSUM"))
        ps_po = attn_ctx.enter_context(tc.tile_pool(name="ps_po", bufs=1, space="PSUM"))
        ps_rtb = attn_ctx.enter_context(tc.tile_pool(name="ps_rtb", bufs=1, space="PSUM"))
        ps_fill = attn_ctx.enter_context(tc.tile_pool(name="ps_fill", bufs=1, space="PSUM"))

        fill_queue = []

        def pop_fill(n):
            # pop up to n PE-filler ops; a DVE op ends the slot so the next
            # PE op that reuses its PSUM bank lands a slot later (no WAR
            # stall); DMA ops are free
            got = 0
            while fill_queue and got < n:
                kind, f = fill_queue.pop(0)
                f()
                if kind == "pe":
                    got += 1
                elif kind == "dve":
                    break

        for hf in range(2):
            for p in range(HP):
                if hf == 0:
                    pop_fill(len(fill_queue))  # qt(p) must be complete
                    if p + 1 < HP:
                        fill_queue.extend(qproj_ops(p + 1, ps_fill, "fill"))
                else:
                    if p % 2 == 0:
                        fill_queue.extend(
                            outproj_ops(p // 2, ps_fill, "fill", out_sb))
                q0 = p * NQ + hf * 512
                po0 = ps_po.tile([P, 512], F32, tag="po0", name=f"po0_{hf}_{p}")
                po1 = ps_po.tile([P, 512], F32, tag="po1", name=f"po1_{hf}_{p}")
                esl = []

                def S(c, p=p, q0=q0, esl=esl):
                    ps = ps_s.tile([P, NQ], F32, tag="ss", name=f"ss{hf}_{p}_{c}")
                    for hh in range(2):
                        nc.tensor.matmul(
                            ps[:, hh * 512:(hh + 1) * 512],
                            kt[hh * D:(hh + 1) * D,
                               p * NKVP + c * P: p * NKVP + (c + 1) * P],
                            qt[hh * D:(hh + 1) * D, q0:q0 + 512],
                            start=True, stop=True,
                            tile_position=(hh * D, 0))
                    es = es_pool.tile([P, NQ], BF16, tag="es",
                                      name=f"es{hf}_{p}_{c}")
                    nc.scalar.activation(es[:], ps[:], AF.Exp, scale=float(SCALE))
                    esl.append(es)

                def AV(c, p=p, po0=po0, po1=po1, esl=esl):
                    h0, h1 = 2 * p, 2 * p + 1
                    nc.tensor.matmul(
                        po0[0:VW, :],
                        vt[:, c * CW + h0 * VW: c * CW + (h0 + 1) * VW],
                        esl[c][:, 0:512],
                        start=(c == 0), stop=(c == nkc - 1),
                        skip_group_check=True)
                    nc.tensor.matmul(
                        po1[0:VW, :],
                        vt[:, c * CW + h1 * VW: c * CW + (h1 + 1) * VW],
                        esl[c][:, 512:1024],
                        start=(c == 0), stop=(c == nkc - 1),
                        skip_group_check=True)

                S(0)
                if nkc > 1:
                    S(1)
                pop_fill((6 if p == 0 else 4) if hf == 0 else 1)
                for c in range(nkc):
                    AV(c)
                    if c + 2 < nkc:
                        S(c + 2)
                    pop_fill(2 if hf == 0 else (1 if c < 3 else 0))
                # epilogue: normalize this (pair, nq-half) slice. po -> SBUF
                # copies come first so the po banks free early (next pair's
                # AV(0) is off the rcp->bcast->mul critical chain); the muls
                # are then SBUF-only bf16 (fast DVE mode)
                rcp = rcp_pool.tile([P, NQ], BF16, tag="rcp", name=f"rcp{hf}_{p}")
                posb = rcp_pool.tile([P, 512], BF16, tag="posb",
                                     name=f"posb{hf}_{p}")
                rtb_sb = rcp_pool.tile([P, 512], BF16, tag="rtbs",
                                       name=f"rtbs{hf}_{p}")
                with nc.allow_low_precision(reason="softmax reciprocal"):
                    nc.vector.reciprocal(rcp[D:D + 1, 0:512], po0[D:D + 1, :])
                    nc.vector.reciprocal(rcp[D:D + 1, 512:1024], po1[D:D + 1, :])
                # (DVE, not ScalarE: an AF.Copy between exps would reload the
                # activation table on HW, ~1.3us per switch)
                nc.vector.tensor_copy(posb[0:D, :], po0[0:D, :])
                nc.vector.tensor_copy(posb[D:P, :], po1[0:D, :])
                rtb = ps_rtb.tile([P, 512], F32, tag="rtb", name=f"rtb{hf}_{p}")
                pop_fill(1)
                nc.tensor.matmul(rtb[0:D, :], ones[D:D + 1, 0:D],
                                 rcp[D:D + 1, 0:512], start=True, stop=True,
                                 skip_group_check=True)
                pop_fill(1)
                nc.tensor.matmul(rtb[D:P, :], ones[D:D + 1, 0:D],
                                 rcp[D:D + 1, 512:1024], start=True, stop=True,
                                 skip_group_check=True)
                pop_fill(1)
                nc.vector.tensor_copy(rtb_sb[:], rtb[:])
                nc.vector.tensor_mul(ot[:, q0:q0 + 512], posb[:], rtb_sb[:])
                pop_fill(3 if hf == 0 else 4)

        # ---- trailing output projection (own double-buffered PSUM) ----
        while fill_queue:
            pop_fill(len(fill_queue))
        attn_ctx.close()
        tail_ctx = ExitStack()
        ps_tail = tail_ctx.enter_context(
            tc.tile_pool(name="ps_tail", bufs=2, space="PSUM"))
        for m in range(NQC // 2, NQC):
            for _, f in outproj_ops(m, ps_tail, "tail", out_sb):
                f()
        tail_ctx.close()


_CACHED = {}


def _build(iters=1, loop=1, nkc=5, stages=("proj", "attn", "out"), staggered=False):
    """Build the program. `iters` unrolls the body in the instruction stream;
    `loop` wraps it in an on-device hardware loop (constant program size) —
    used by test.py to measure per-body device time as a slope. `nkc` is the
    number of 128-row kv chunks after mask compaction. `stages` is kept for
    test.py compatibility (cache key only — the full kernel is always
    emitted; bias costs nothing extra now)."""
    key = (iters, loop, nkc, tuple(stages), staggered)
    if key in _CACHED:
        return _CACHED[key]
    NKVP = nkc * P
    nc = bacc.Bacc("TRN2", debug=False, target_bir_lowering=False)
    xt = nc.dram_tensor("xt", [QD, NQ], BF16, kind="ExternalInput").ap()
    ctx_t = nc.dram_tensor("ctx_t", [CD, NKVP], BF16, kind="ExternalInput").ap()
    mask64 = nc.dram_tensor("mask64", [P, nkc * D], BF16,
                            kind="ExternalInput").ap()
    wq_d = nc.dram_tensor("wq", [QD, INNER], BF16, kind="ExternalInput").ap()
    wk_d = nc.dram_tensor("wk", [CD, INNER], BF16, kind="ExternalInput").ap()
    wv_d = nc.dram_tensor("wv", [CD, INNER], BF16, kind="ExternalInput").ap()
    wo_d = nc.dram_tensor("wo", [INNER, QD], BF16, kind="ExternalInput").ap()
    bob_d = nc.dram_tensor("bob", [P, QD], BF16, kind="ExternalInput").ap()
    out_d = nc.dram_tensor("out", [NQ, QD], BF16, kind="ExternalOutput").ap()
    io = (xt, ctx_t, mask64, wq_d, wk_d, wv_d, wo_d, bob_d, out_d)
    with tile.TileContext(nc) as tc:
        if loop > 1:
            with tc.For_i(0, loop, 1, staggered_reset=staggered,
                          hint_engines=(mybir.EngineType.PE,)):
                for _ in range(iters):
                    _emit(tc, io, nkc, stages)
        else:
            for _ in range(iters):
                _emit(tc, io, nkc, stages)
    nc.compile()
    _CACHED[key] = nc
    return nc


def make_in_maps(x, context, mask, Wq, Wk, Wv, Wo, bo):
    x = np.asarray(x, dtype=np.float32)
    context = np.asarray(context, dtype=np.float32)
    mask_b = np.asarray(mask).astype(bool)
    Wq = np.ascontiguousarray(np.asarray(Wq, dtype=np.float32)).astype(NPBF16)
    Wk = np.ascontiguousarray(np.asarray(Wk, dtype=np.float32)).astype(NPBF16)
    Wv = np.ascontiguousarray(np.asarray(Wv, dtype=np.float32)).astype(NPBF16)
    Wo = np.ascontiguousarray(np.asarray(Wo, dtype=np.float32)).astype(NPBF16)
    bo = np.asarray(bo, dtype=np.float32)
    bob = np.ascontiguousarray(
        np.broadcast_to(bo[None, :], (P, QD))).astype(NPBF16)

    counts = mask_b.sum(axis=1)
    n_max = max(int(counts.max()), 1)
    nkc = (n_max + P - 1) // P
    NKVP = nkc * P

    # a fully-masked batch reduces to uniform attention over all keys:
    # emulate exactly by sending the full context unmasked with Wq zeroed
    # (s = 0 -> softmax uniform), matching the reference's -inf softmax
    if (counts == 0).any():
        nkc = NKV // P
        NKVP = nkc * P

    ctx_ts, m64s, wq_zero = [], [], []
    for b in range(4):
        idx = np.nonzero(mask_b[b])[0]
        n = len(idx)
        wq_zero.append(n == 0)
        ctx_c = np.zeros((NKVP, CD), np.float32)
        if n:
            ctx_c[:n] = context[b][idx]
        else:
            n = NKV
            ctx_c[:n] = context[b]
        ctx_ts.append(np.ascontiguousarray(ctx_c.T).astype(NPBF16))
        valid = (np.arange(NKVP) < n).reshape(nkc, P)      # [c, p]
        m64 = np.repeat(valid.T[:, :, None], D, axis=2)    # [p, c, 64]
        m64s.append(np.ascontiguousarray(
            m64.reshape(P, nkc * D)).astype(NPBF16))

    in_maps = []
    for b in range(4):
        for qh in range(2):
            in_maps.append({
                "xt": np.ascontiguousarray(
                    x[b, qh * NQ:(qh + 1) * NQ, :].T).astype(NPBF16),
                "ctx_t": ctx_ts[b],
                "mask64": m64s[b],
                "wq": np.zeros_like(Wq) if wq_zero[b] else Wq,
                "wk": Wk, "wv": Wv, "wo": Wo, "bob": bob,
            })
    return in_maps, nkc


def run_sharded(x, context, mask, Wq, Wk, Wv, Wo, bo, trace=False, **kw):
    in_maps, nkc = make_in_maps(x, context, mask, Wq, Wk, Wv, Wo, bo)
    stages = ("proj", "attn", "out") + (
        () if np.asarray(bo).any() else ("nobias",))
    nc = _build(nkc=nkc, stages=stages)
    res = run_bass_kernel_spmd(nc, in_maps, list(range(8)), trace=trace, **kw)
    out = np.empty((4, 2 * NQ, QD), dtype=np.float32)
    for i in range(8):
        b, qh = divmod(i, 2)
        out[b, qh * NQ:(qh + 1) * NQ, :] = res.results[i]["out"].astype(
            np.float32)
    return out, res


def kernel(x, context, mask, Wq, Wk, Wv, Wo, bo):
    out, _ = run_sharded(x, context, mask, Wq, Wk, Wv, Wo, bo, trace=False)
    return out


# revision 34
# speedup vs baseline: 1.1640x; 1.1640x over previous
"""Trainium2 Bass kernel for cross-attention (b=4, nq=2048, nkv=1024,
qdim=1024, cdim=768, heads=16, dim_head=64).

Sharding: 8 cores = batch(4) x nq-half(2). Each core computes a disjoint
[1024, 1024] slice of the output; no collectives needed.

Host-side prep (part of kernel()):
  - KV compaction: the per-key mask drops ~half the keys. Gather the
    unmasked context rows per batch, pad with zeros to a multiple of 128
    (NKVP = nkc*128 kv positions). softmax over masked entries ==
    softmax over the unmasked subset, so this is exact.
  - x and ctx are uploaded pre-transposed (xt = x^T, ctx_t = ctx^T), which
    removes all PE transpose work on device.
  - mask64 ([128, nkc*64], bf16): per-kv-chunk validity column-broadcast
    to 64 cols; used as matmul weights for the softmax rowsum so padding
    rows are excluded. Pad rows of ctx are zero so V pad rows are zero.

Per-core device algorithm:
  KT = Wk^T @ CT   [inner, NKVP]   (inner chunk p holds heads 2p, 2p+1)
  V  = CT^T @ Wv   [NKVP, inner]   (pad rows zero by construction)
  QT = Wq^T @ XT   [inner, nq]     (single pass, all 8 Wq chunks resident)
  per head-pair p, per q-half hf, per kv-chunk c:
    S^T = K_h @ Q_h^T              (row-tiled K=64: 2 heads concurrent)
    ES  = exp(SCALE * S^T)         (ScalarE, PSUM->SBUF, bf16)
    OT_acc  += V_h^T @ ES          (col-tiled M=64: 2 heads concurrent)
    rs_acc  += mask64^T @ ES       (col-tiled M=64, excludes pad rows)
  rT = 1/rs ; OT = OT_acc * rT (bf16)
  out = (OT^T stacked) @ Wo + 1 x bo  (bf16 matmuls, bias via K=1 matmul)

Scores are O(1) by construction so unshifted exp is safe.
"""

import numpy as np
from contextlib import ExitStack

import ml_dtypes

import concourse.bass as bass
import concourse.mybir as mybir
import concourse.tile as tile
from concourse import bacc
from concourse.bass_utils import run_bass_kernel_spmd

F32 = mybir.dt.float32
F32R = mybir.dt.float32r
BF16 = mybir.dt.bfloat16
AF = mybir.ActivationFunctionType
NPBF16 = ml_dtypes.bfloat16

NQ = 1024      # queries per core
NKV = 1024
QD = 1024
CD = 768
H = 16
D = 64
INNER = 1024
SCALE = D ** -0.5
P = 128
NQC = NQ // P      # 8 nq chunks
QDC = QD // P      # 8
CDC = CD // P      # 6
HP = H // 2        # 8 head pairs


def R(ap):
    return ap.bitcast(F32R)


def _emit(tc, io, nkc, stages=("proj", "attn", "out")):
    nc = tc.nc
    xt_d, ctx_t, mask64_d, wq_d, wk_d, wv_d, wo_d, bo_d, out_d = io
    NKVP = nkc * P
    do_proj = "proj" in stages
    attn_lv = (4 if "attn" in stages else 3 if "attn3" in stages
               else 2 if "attn2" in stages else 1 if "attn1" in stages else 0)
    do_out = "out" in stages
    has_bias = "nobias" not in stages

    with ExitStack() as top:
        # ---- flat pools: everything resident, no pool-boundary barriers ----
        const = top.enter_context(tc.tile_pool(name="const", bufs=1))
        big = top.enter_context(tc.tile_pool(name="big", bufs=1))

        ones = const.tile([1, P], BF16, tag="ones")
        nc.vector.memset(ones[:], 1.0)
        mask64 = const.tile([P, nkc * D], BF16, tag="mask64")
        nc.sync.dma_start(out=mask64[:], in_=mask64_d)

        ot = big.tile([P, QDC * NQ], BF16, tag="ot")     # O^T: chunk k cols k*NQ..
        qt = big.tile([P, HP * NQ], BF16, tag="qt")      # Q^T: chunk p cols p*NQ..
        kt = big.tile([P, HP * NKVP], BF16, tag="kt")
        vt = big.tile([P, nkc * INNER], BF16, tag="vt")  # V: chunk c cols c*INNER..

        # ---- input loads (few big DMAs, split across both HWDGE queues;
        # ordered so K-proj deps (ct on sync, wk on scalar) land first) ----
        ct = big.tile([P, CDC * NKVP], BF16, tag="ct")
        ct3 = ct.rearrange("p (k n) -> p k n", n=NKVP)
        nc.sync.dma_start(out=ct3[:], in_=ctx_t.rearrange("(k p) n -> p k n", p=P))
        wkt = big.tile([P, CDC * INNER], BF16, tag="wk")
        wk3 = wkt.rearrange("p (k n) -> p k n", n=INNER)
        nc.scalar.dma_start(out=wk3[:], in_=wk_d.rearrange("(k p) n -> p k n", p=P))
        wk = [wk3[:, k, :] for k in range(CDC)]
        # wv before xt/wq: the single DMA resource serializes all queues, so
        # V-proj's dependency must not queue behind Q-proj's inputs
        wvt = big.tile([P, CDC * INNER], BF16, tag="wv")
        wv3 = wvt.rearrange("p (k n) -> p k n", n=INNER)
        nc.sync.dma_start(out=wv3[:], in_=wv_d.rearrange("(k p) n -> p k n", p=P))
        wv = [wv3[:, k, :] for k in range(CDC)]
        xt = big.tile([P, QDC * NQ], BF16, tag="xt")
        xt3 = xt.rearrange("p (k n) -> p k n", n=NQ)
        nc.sync.dma_start(out=xt3[:], in_=xt_d.rearrange("(k p) n -> p k n", p=P))
        wqt = big.tile([P, QDC * INNER], BF16, tag="wq")
        wq3 = wqt.rearrange("p (k n) -> p k n", n=INNER)
        nc.scalar.dma_start(out=wq3[:], in_=wq_d.rearrange("(k p) n -> p k n", p=P))
        wq = [wq3[:, k, :] for k in range(QDC)]
        wo = big.tile([P, QDC * QD], BF16, tag="wo")
        wo3 = wo.rearrange("p (k n) -> p k n", n=QD)
        nc.scalar.dma_start(out=wo3[:], in_=wo_d.rearrange("(k p) n -> p k n", p=P))
        bo_t = const.tile([1, QD], BF16, tag="bo")
        nc.sync.dma_start(out=bo_t[:], in_=bo_d[:].rearrange("(o n) -> o n", o=1))

        # ---- K projection: KT[inner, kv] (psum must not cross 2KB banks) ----
        ksplits = [(lo, min(lo + 512, NKVP)) for lo in range(0, NKVP, 512)]
        proj_ctx = ExitStack()
        pj_ps = proj_ctx.enter_context(tc.tile_pool(name="pj_ps", bufs=2, space="PSUM"))
        for p in range(HP if do_proj else 0):
            ps = pj_ps.tile([P, NKVP], F32, tag="pj", name=f"pjk{p}")
            for k in range(CDC):
                for (lo, hi) in ksplits:
                    nc.tensor.matmul(
                        ps[:, lo:hi], wk[k][:, p * P:(p + 1) * P],
                        ct[:, k * NKVP + lo: k * NKVP + hi],
                        start=(k == 0), stop=(k == CDC - 1),
                        skip_group_check=True)
            nc.vector.tensor_copy(kt[:, p * NKVP:(p + 1) * NKVP], ps[:])

        # ---- V projection ----
        vq_ps = proj_ctx.enter_context(tc.tile_pool(name="vq_ps", bufs=4, space="PSUM"))
        for c in range(nkc if do_proj else 0):
            for hf in range(2):
                ps = vq_ps.tile([P, 512], F32, tag="vq", name=f"pjv{c}_{hf}")
                for k in range(CDC):
                    nc.tensor.matmul(
                        ps[:], ct[:, k * NKVP + c * P: k * NKVP + (c + 1) * P],
                        wv[k][:, hf * 512:(hf + 1) * 512],
                        start=(k == 0), stop=(k == CDC - 1))
                nc.vector.tensor_copy(
                    vt[:, c * INNER + hf * 512: c * INNER + (hf + 1) * 512],
                    ps[:])

        # ---- Q projection ----
        for p in range(HP if do_proj else 0):
            for hf in range(2):
                ps = vq_ps.tile([P, 512], F32, tag="vq", name=f"pjq{p}_{hf}")
                for k in range(QDC):
                    nc.tensor.matmul(
                        ps[:], wq[k][:, p * P:(p + 1) * P],
                        xt[:, k * NQ + hf * 512: k * NQ + (hf + 1) * 512],
                        start=(k == 0), stop=(k == QDC - 1))
                nc.vector.tensor_copy(
                    qt[:, p * NQ + hf * 512: p * NQ + (hf + 1) * 512], ps[:])

        # ---- attention ----
        proj_ctx.close()
        attn_ctx = ExitStack()
        es_pool = attn_ctx.enter_context(tc.tile_pool(name="esp", bufs=5))
        rt_pool = attn_ctx.enter_context(tc.tile_pool(name="rtp", bufs=2))
        ps_s = attn_ctx.enter_context(tc.tile_pool(name="ps_s", bufs=3, space="PSUM"))
        ps_o = attn_ctx.enter_context(tc.tile_pool(name="ps_o", bufs=1, space="PSUM"))
        ps_r = attn_ctx.enter_context(tc.tile_pool(name="ps_r", bufs=1, space="PSUM"))
        for p in range(HP if attn_lv else 0):
            for hf in range(2):
                po = ps_o.tile([P, 512], F32, tag="po", name=f"po{p}_{hf}")
                pr = ps_r.tile([P, 512], F32, tag="pr", name=f"pr{p}_{hf}")
                for c in range(nkc):
                    # S^T for both heads of the pair (row-tiled K=64):
                    # head h -> cols 0:512, head h\' -> cols 512:1024
                    ps = ps_s.tile([P, NQ], F32, tag="ss", name=f"ss{p}_{hf}_{c}")
                    for hh in range(2):
                        nc.tensor.matmul(
                            ps[:, hh * 512:(hh + 1) * 512],
                            kt[hh * D:(hh + 1) * D,
                               p * NKVP + c * P: p * NKVP + (c + 1) * P],
                            qt[hh * D:(hh + 1) * D,
                               p * NQ + hf * 512: p * NQ + (hf + 1) * 512],
                            start=True, stop=True,
                            tile_position=(hh * D, 0))
                    if attn_lv < 2:
                        continue
                    es = es_pool.tile([P, NQ], BF16, tag="es",
                                      name=f"es{p}_{hf}_{c}")
                    nc.scalar.activation(es[:], ps[:], AF.Exp, scale=float(SCALE))
                    for hh in range(2 if attn_lv >= 3 else 0):
                        h = 2 * p + hh
                        esl = es[:, hh * 512:(hh + 1) * 512]
                        nc.tensor.matmul(
                            po[hh * D:(hh + 1) * D, :],
                            vt[:, c * INNER + h * D: c * INNER + (h + 1) * D],
                            esl,
                            start=(c == 0), stop=(c == nkc - 1),
                            tile_position=(0, hh * D),
                            skip_group_check=True)
                        if attn_lv >= 4:
                            nc.tensor.matmul(
                                pr[hh * D:(hh + 1) * D, :],
                                mask64[:, c * D:(c + 1) * D], esl,
                                start=(c == 0), stop=(c == nkc - 1),
                                tile_position=(0, hh * D),
                                skip_group_check=True)
                if attn_lv < 4:
                    continue
                # epilogue: normalize this (pair, nq-half) slice
                rt = rt_pool.tile([P, 512], F32, tag="rt", name=f"rt{p}_{hf}")
                with nc.allow_low_precision(reason="softmax reciprocal"):
                    nc.vector.reciprocal(rt[:], pr[:])
                nc.vector.tensor_mul(
                    ot[:, p * NQ + hf * 512: p * NQ + (hf + 1) * 512],
                    po[:], rt[:])


        # ---- output projection ----
        attn_ctx.close()
        out_ps = top.enter_context(tc.tile_pool(name="out_ps", bufs=6, space="PSUM"))
        out_sb = top.enter_context(tc.tile_pool(name="out_sb", bufs=3))
        for m in range(NQC if do_out else 0):
            sb = out_sb.tile([P, QD], BF16, tag="osb", name=f"osb{m}")
            for n in range(2):
                ps = out_ps.tile([P, 512], F32, tag="ops", name=f"ops{m}_{n}")
                for k in range(QDC):
                    nc.tensor.matmul(
                        ps[:],
                        ot[:, k * NQ + m * P: k * NQ + (m + 1) * P],
                        wo3[:, k, n * 512:(n + 1) * 512],
                        start=(k == 0),
                        stop=(k == QDC - 1) and not has_bias,
                        skip_group_check=True)
                if has_bias:
                    nc.tensor.matmul(
                        ps[:], ones[0:1, 0:P],
                        bo_t[0:1, n * 512:(n + 1) * 512],
                        start=False, stop=True, skip_group_check=True)
                nc.vector.tensor_copy(sb[:, n * 512:(n + 1) * 512], ps[:])
            nc.sync.dma_start(out=out_d[m * P:(m + 1) * P, :], in_=sb[:])

_CACHED = {}


def _build(iters=1, loop=1, nkc=5, stages=("proj", "attn", "out"), staggered=False):
    """Build the program. `iters` unrolls the body in the instruction stream;
    `loop` wraps it in an on-device hardware loop (constant program size) —
    used by test.py to measure per-body device time as a slope. `nkc` is the
    number of 128-row kv chunks after mask compaction. `stages` restricts the
    emitted phases (timing probes only — output is garbage unless full)."""
    key = (iters, loop, nkc, tuple(stages), staggered)
    if key in _CACHED:
        return _CACHED[key]
    NKVP = nkc * P
    nc = bacc.Bacc("TRN2", debug=False, target_bir_lowering=False)
    xt = nc.dram_tensor("xt", [QD, NQ], BF16, kind="ExternalInput").ap()
    ctx_t = nc.dram_tensor("ctx_t", [CD, NKVP], BF16, kind="ExternalInput").ap()
    mask64 = nc.dram_tensor("mask64", [P, nkc * D], BF16,
                            kind="ExternalInput").ap()
    wq_d = nc.dram_tensor("wq", [QD, INNER], BF16, kind="ExternalInput").ap()
    wk_d = nc.dram_tensor("wk", [CD, INNER], BF16, kind="ExternalInput").ap()
    wv_d = nc.dram_tensor("wv", [CD, INNER], BF16, kind="ExternalInput").ap()
    wo_d = nc.dram_tensor("wo", [INNER, QD], BF16, kind="ExternalInput").ap()
    bo_d = nc.dram_tensor("bo", [QD], BF16, kind="ExternalInput").ap()
    out_d = nc.dram_tensor("out", [NQ, QD], BF16, kind="ExternalOutput").ap()
    io = (xt, ctx_t, mask64, wq_d, wk_d, wv_d, wo_d, bo_d, out_d)
    with tile.TileContext(nc) as tc:
        if loop > 1:
            with tc.For_i(0, loop, 1, staggered_reset=staggered,
                          hint_engines=(mybir.EngineType.PE,)):
                for _ in range(iters):
                    _emit(tc, io, nkc, stages)
        else:
            for _ in range(iters):
                _emit(tc, io, nkc, stages)
    nc.compile()
    _CACHED[key] = nc
    return nc


def make_in_maps(x, context, mask, Wq, Wk, Wv, Wo, bo):
    x = np.asarray(x, dtype=np.float32)
    context = np.asarray(context, dtype=np.float32)
    mask_b = np.asarray(mask).astype(bool)
    Wq = np.ascontiguousarray(np.asarray(Wq, dtype=np.float32)).astype(NPBF16)
    Wk = np.ascontiguousarray(np.asarray(Wk, dtype=np.float32)).astype(NPBF16)
    Wv = np.ascontiguousarray(np.asarray(Wv, dtype=np.float32)).astype(NPBF16)
    Wo = np.ascontiguousarray(np.asarray(Wo, dtype=np.float32)).astype(NPBF16)
    bo = np.ascontiguousarray(np.asarray(bo, dtype=np.float32)).astype(NPBF16)

    counts = mask_b.sum(axis=1)
    n_max = max(int(counts.max()), 1)
    nkc = (n_max + P - 1) // P
    NKVP = nkc * P

    # a fully-masked batch reduces to uniform attention over all keys:
    # emulate exactly by sending the full context unmasked with Wq zeroed
    # (s = 0 -> softmax uniform), matching the reference's -inf softmax
    if (counts == 0).any():
        nkc = NKV // P
        NKVP = nkc * P

    ctx_ts, m64s, wq_zero = [], [], []
    for b in range(4):
        idx = np.nonzero(mask_b[b])[0]
        n = len(idx)
        wq_zero.append(n == 0)
        ctx_c = np.zeros((NKVP, CD), np.float32)
        if n:
            ctx_c[:n] = context[b][idx]
        else:
            n = NKV
            ctx_c[:n] = context[b]
        ctx_ts.append(np.ascontiguousarray(ctx_c.T).astype(NPBF16))
        valid = (np.arange(NKVP) < n).reshape(nkc, P)      # [c, p]
        m64 = np.repeat(valid.T[:, :, None], D, axis=2)    # [p, c, 64]
        m64s.append(np.ascontiguousarray(
            m64.reshape(P, nkc * D)).astype(NPBF16))

    in_maps = []
    for b in range(4):
        for qh in range(2):
            in_maps.append({
                "xt": np.ascontiguousarray(
                    x[b, qh * NQ:(qh + 1) * NQ, :].T).astype(NPBF16),
                "ctx_t": ctx_ts[b],
                "mask64": m64s[b],
                "wq": np.zeros_like(Wq) if wq_zero[b] else Wq,
                "wk": Wk, "wv": Wv, "wo": Wo, "bo": bo,
            })
    return in_maps, nkc


def run_sharded(x, context, mask, Wq, Wk, Wv, Wo, bo, trace=False, **kw):
    in_maps, nkc = make_in_maps(x, context, mask, Wq, Wk, Wv, Wo, bo)
    stages = ("proj", "attn", "out") + (
        () if np.asarray(bo).any() else ("nobias",))
    nc = _build(nkc=nkc, stages=stages)
    res = run_bass_kernel_spmd(nc, in_maps, list(range(8)), trace=trace, **kw)
    out = np.empty((4, 2 * NQ, QD), dtype=np.float32)
    for i in range(8):
        b, qh = divmod(i, 2)
        out[b, qh * NQ:(qh + 1) * NQ, :] = res.results[i]["out"].astype(np.float32)
    return out, res


def kernel(x, context, mask, Wq, Wk, Wv, Wo, bo):
    out, _ = run_sharded(x, context, mask, Wq, Wk, Wv, Wo, bo, trace=False)
    return out



# revision 36
# speedup vs baseline: 1.1791x; 1.0130x over previous
"""Trainium2 Bass kernel for cross-attention (b=4, nq=2048, nkv=1024,
qdim=1024, cdim=768, heads=16, dim_head=64).

Sharding: 8 cores = batch(4) x nq-half(2). Each core computes a disjoint
[1024, 1024] slice of the output; no collectives needed.

Host-side prep (part of kernel()):
  - KV compaction: the per-key mask drops ~half the keys. Gather the
    unmasked context rows per batch, pad with zeros to a multiple of 128
    (NKVP = nkc*128 kv positions). softmax over masked entries ==
    softmax over the unmasked subset, so this is exact.
  - x and ctx are uploaded pre-transposed (xt = x^T, ctx_t = ctx^T), which
    removes all PE transpose work on device.
  - mask64 ([128, nkc*64], bf16): per-kv-chunk validity column-broadcast
    to 64 cols; used as matmul weights for the softmax rowsum so padding
    rows are excluded. Pad rows of ctx are zero so V pad rows are zero.

Per-core device algorithm:
  KT = Wk^T @ CT   [inner, NKVP]   (inner chunk p holds heads 2p, 2p+1)
  V  = CT^T @ Wv   [NKVP, inner]   (pad rows zero by construction)
  QT = Wq^T @ XT   [inner, nq]     (single pass, all 8 Wq chunks resident)
  per head-pair p, per q-half hf, per kv-chunk c:
    S^T = K_h @ Q_h^T              (row-tiled K=64: 2 heads concurrent)
    ES  = exp(SCALE * S^T)         (ScalarE, PSUM->SBUF, bf16)
    OT_acc  += V_h^T @ ES          (col-tiled M=64: 2 heads concurrent)
    rs_acc  += mask64^T @ ES       (col-tiled M=64, excludes pad rows)
  rT = 1/rs ; OT = OT_acc * rT (bf16)
  out = (OT^T stacked) @ Wo + 1 x bo  (bf16 matmuls, bias via K=1 matmul)

Scores are O(1) by construction so unshifted exp is safe.
"""

import numpy as np
from contextlib import ExitStack

import ml_dtypes

import concourse.bass as bass
import concourse.mybir as mybir
import concourse.tile as tile
from concourse import bacc
from concourse.bass_utils import run_bass_kernel_spmd

F32 = mybir.dt.float32
F32R = mybir.dt.float32r
BF16 = mybir.dt.bfloat16
AF = mybir.ActivationFunctionType
NPBF16 = ml_dtypes.bfloat16

NQ = 1024      # queries per core
NKV = 1024
QD = 1024
CD = 768
H = 16
D = 64
INNER = 1024
SCALE = D ** -0.5
P = 128
NQC = NQ // P      # 8 nq chunks
QDC = QD // P      # 8
CDC = CD // P      # 6
HP = H // 2        # 8 head pairs


def R(ap):
    return ap.bitcast(F32R)


def _emit(tc, io, nkc, stages=("proj", "attn", "out")):
    nc = tc.nc
    xt_d, ctx_t, mask64_d, wq_d, wk_d, wv_d, wo_d, bo_d, out_d = io
    NKVP = nkc * P
    do_proj = "proj" in stages
    attn_lv = (4 if "attn" in stages else 3 if "attn3" in stages
               else 2 if "attn2" in stages else 1 if "attn1" in stages else 0)
    do_out = "out" in stages
    has_bias = "nobias" not in stages

    with ExitStack() as top:
        # ---- flat pools: everything resident, no pool-boundary barriers ----
        const = top.enter_context(tc.tile_pool(name="const", bufs=1))
        big = top.enter_context(tc.tile_pool(name="big", bufs=1))

        ones = const.tile([1, P], BF16, tag="ones")
        nc.vector.memset(ones[:], 1.0)
        mask64 = const.tile([P, nkc * D], BF16, tag="mask64")
        nc.sync.dma_start(out=mask64[:], in_=mask64_d)

        ot = big.tile([P, QDC * NQ], BF16, tag="ot")     # O^T: chunk k cols k*NQ..
        qt = big.tile([P, HP * NQ], BF16, tag="qt")      # Q^T: chunk p cols p*NQ..
        kt = big.tile([P, HP * NKVP], BF16, tag="kt")
        vt = big.tile([P, nkc * INNER], BF16, tag="vt")  # V: chunk c cols c*INNER..

        # ---- input loads (few big DMAs, split across both HWDGE queues;
        # ordered so K-proj deps (ct on sync, wk on scalar) land first) ----
        ct = big.tile([P, CDC * NKVP], BF16, tag="ct")
        ct3 = ct.rearrange("p (k n) -> p k n", n=NKVP)
        nc.sync.dma_start(out=ct3[:], in_=ctx_t.rearrange("(k p) n -> p k n", p=P))
        wkt = big.tile([P, CDC * INNER], BF16, tag="wk")
        wk3 = wkt.rearrange("p (k n) -> p k n", n=INNER)
        nc.scalar.dma_start(out=wk3[:], in_=wk_d.rearrange("(k p) n -> p k n", p=P))
        wk = [wk3[:, k, :] for k in range(CDC)]
        xt = big.tile([P, QDC * NQ], BF16, tag="xt")
        xt3 = xt.rearrange("p (k n) -> p k n", n=NQ)
        nc.sync.dma_start(out=xt3[:], in_=xt_d.rearrange("(k p) n -> p k n", p=P))
        wqt = big.tile([P, QDC * INNER], BF16, tag="wq")
        wq3 = wqt.rearrange("p (k n) -> p k n", n=INNER)
        nc.scalar.dma_start(out=wq3[:], in_=wq_d.rearrange("(k p) n -> p k n", p=P))
        wq = [wq3[:, k, :] for k in range(QDC)]
        wvt = big.tile([P, CDC * INNER], BF16, tag="wv")
        wv3 = wvt.rearrange("p (k n) -> p k n", n=INNER)
        nc.sync.dma_start(out=wv3[:], in_=wv_d.rearrange("(k p) n -> p k n", p=P))
        wv = [wv3[:, k, :] for k in range(CDC)]
        wo = big.tile([P, QDC * QD], BF16, tag="wo")
        wo3 = wo.rearrange("p (k n) -> p k n", n=QD)
        nc.scalar.dma_start(out=wo3[:], in_=wo_d.rearrange("(k p) n -> p k n", p=P))
        bo_t = const.tile([1, QD], BF16, tag="bo")
        nc.sync.dma_start(out=bo_t[:], in_=bo_d[:].rearrange("(o n) -> o n", o=1))

        # ---- K projection: KT[inner, kv] (psum must not cross 2KB banks) ----
        ksplits = [(lo, min(lo + 512, NKVP)) for lo in range(0, NKVP, 512)]
        proj_ctx = ExitStack()
        pj_ps = proj_ctx.enter_context(tc.tile_pool(name="pj_ps", bufs=2, space="PSUM"))
        for p in range(HP if do_proj else 0):
            ps = pj_ps.tile([P, NKVP], F32, tag="pj", name=f"pjk{p}")
            for k in range(CDC):
                for (lo, hi) in ksplits:
                    nc.tensor.matmul(
                        ps[:, lo:hi], wk[k][:, p * P:(p + 1) * P],
                        ct[:, k * NKVP + lo: k * NKVP + hi],
                        start=(k == 0), stop=(k == CDC - 1),
                        skip_group_check=True)
            nc.vector.tensor_copy(kt[:, p * NKVP:(p + 1) * NKVP], ps[:])

        # ---- V projection ----
        vq_ps = proj_ctx.enter_context(tc.tile_pool(name="vq_ps", bufs=4, space="PSUM"))
        for c in range(nkc if do_proj else 0):
            for hf in range(2):
                ps = vq_ps.tile([P, 512], F32, tag="vq", name=f"pjv{c}_{hf}")
                for k in range(CDC):
                    nc.tensor.matmul(
                        ps[:], ct[:, k * NKVP + c * P: k * NKVP + (c + 1) * P],
                        wv[k][:, hf * 512:(hf + 1) * 512],
                        start=(k == 0), stop=(k == CDC - 1))
                nc.vector.tensor_copy(
                    vt[:, c * INNER + hf * 512: c * INNER + (hf + 1) * 512],
                    ps[:])

        # ---- Q projection ----
        for p in range(HP if do_proj else 0):
            for hf in range(2):
                ps = vq_ps.tile([P, 512], F32, tag="vq", name=f"pjq{p}_{hf}")
                for k in range(QDC):
                    nc.tensor.matmul(
                        ps[:], wq[k][:, p * P:(p + 1) * P],
                        xt[:, k * NQ + hf * 512: k * NQ + (hf + 1) * 512],
                        start=(k == 0), stop=(k == QDC - 1))
                nc.vector.tensor_copy(
                    qt[:, p * NQ + hf * 512: p * NQ + (hf + 1) * 512], ps[:])

        # ---- attention ----
        proj_ctx.close()
        attn_ctx = ExitStack()
        es_pool = attn_ctx.enter_context(tc.tile_pool(name="esp", bufs=5))
        rt_pool = attn_ctx.enter_context(tc.tile_pool(name="rtp", bufs=2))
        ps_s = attn_ctx.enter_context(tc.tile_pool(name="ps_s", bufs=3, space="PSUM"))
        ps_o = attn_ctx.enter_context(tc.tile_pool(name="ps_o", bufs=1, space="PSUM"))
        ps_r = attn_ctx.enter_context(tc.tile_pool(name="ps_r", bufs=1, space="PSUM"))
        for p in range(HP if attn_lv else 0):
            for hf in range(2):
                po = ps_o.tile([P, 512], F32, tag="po", name=f"po{p}_{hf}")
                pr = ps_r.tile([P, 512], F32, tag="pr", name=f"pr{p}_{hf}")
                for c in range(nkc):
                    # S^T for both heads of the pair (row-tiled K=64):
                    # head h -> cols 0:512, head h\' -> cols 512:1024
                    ps = ps_s.tile([P, NQ], F32, tag="ss", name=f"ss{p}_{hf}_{c}")
                    for hh in range(2):
                        nc.tensor.matmul(
                            ps[:, hh * 512:(hh + 1) * 512],
                            kt[hh * D:(hh + 1) * D,
                               p * NKVP + c * P: p * NKVP + (c + 1) * P],
                            qt[hh * D:(hh + 1) * D,
                               p * NQ + hf * 512: p * NQ + (hf + 1) * 512],
                            start=True, stop=True,
                            tile_position=(hh * D, 0))
                    if attn_lv < 2:
                        continue
                    es = es_pool.tile([P, NQ], BF16, tag="es",
                                      name=f"es{p}_{hf}_{c}")
                    nc.scalar.activation(es[:], ps[:], AF.Exp, scale=float(SCALE))
                    for hh in range(2 if attn_lv >= 3 else 0):
                        h = 2 * p + hh
                        esl = es[:, hh * 512:(hh + 1) * 512]
                        nc.tensor.matmul(
                            po[hh * D:(hh + 1) * D, :],
                            vt[:, c * INNER + h * D: c * INNER + (h + 1) * D],
                            esl,
                            start=(c == 0), stop=(c == nkc - 1),
                            tile_position=(0, hh * D),
                            skip_group_check=True)
                        if attn_lv >= 4:
                            nc.tensor.matmul(
                                pr[hh * D:(hh + 1) * D, :],
                                mask64[:, c * D:(c + 1) * D], esl,
                                start=(c == 0), stop=(c == nkc - 1),
                                tile_position=(0, hh * D),
                                skip_group_check=True)
                if attn_lv < 4:
                    continue
                # epilogue: normalize this (pair, nq-half) slice
                rt = rt_pool.tile([P, 512], F32, tag="rt", name=f"rt{p}_{hf}")
                with nc.allow_low_precision(reason="softmax reciprocal"):
                    nc.vector.reciprocal(rt[:], pr[:])
                nc.vector.tensor_mul(
                    ot[:, p * NQ + hf * 512: p * NQ + (hf + 1) * 512],
                    po[:], rt[:])


        # ---- output projection ----
        attn_ctx.close()
        out_ps = top.enter_context(tc.tile_pool(name="out_ps", bufs=6, space="PSUM"))
        out_sb = top.enter_context(tc.tile_pool(name="out_sb", bufs=3))
        for m in range(NQC if do_out else 0):
            sb = out_sb.tile([P, QD], F32, tag="osb", name=f"osb{m}")
            for n in range(2):
                ps = out_ps.tile([P, 512], F32, tag="ops", name=f"ops{m}_{n}")
                for k in range(QDC):
                    nc.tensor.matmul(
                        ps[:],
                        ot[:, k * NQ + m * P: k * NQ + (m + 1) * P],
                        wo3[:, k, n * 512:(n + 1) * 512],
                        start=(k == 0),
                        stop=(k == QDC - 1) and not has_bias,
                        skip_group_check=True)
                if has_bias:
                    nc.tensor.matmul(
                        ps[:], ones[0:1, 0:P],
                        bo_t[0:1, n * 512:(n + 1) * 512],
                        start=False, stop=True, skip_group_check=True)
                nc.vector.tensor_copy(sb[:, n * 512:(n + 1) * 512], ps[:])
            nc.sync.dma_start(out=out_d[m * P:(m + 1) * P, :], in_=sb[:])

_CACHED = {}


def _build(iters=1, loop=1, nkc=5, stages=("proj", "attn", "out"), staggered=True):
    """Build the program. `iters` unrolls the body in the instruction stream;
    `loop` wraps it in an on-device hardware loop (constant program size) —
    used by test.py to measure per-body device time as a slope. `nkc` is the
    number of 128-row kv chunks after mask compaction. `stages` restricts the
    emitted phases (timing probes only — output is garbage unless full)."""
    key = (iters, loop, nkc, tuple(stages), staggered)
    if key in _CACHED:
        return _CACHED[key]
    NKVP = nkc * P
    nc = bacc.Bacc("TRN2", debug=False, target_bir_lowering=False)
    xt = nc.dram_tensor("xt", [QD, NQ], BF16, kind="ExternalInput").ap()
    ctx_t = nc.dram_tensor("ctx_t", [CD, NKVP], BF16, kind="ExternalInput").ap()
    mask64 = nc.dram_tensor("mask64", [P, nkc * D], BF16,
                            kind="ExternalInput").ap()
    wq_d = nc.dram_tensor("wq", [QD, INNER], BF16, kind="ExternalInput").ap()
    wk_d = nc.dram_tensor("wk", [CD, INNER], BF16, kind="ExternalInput").ap()
    wv_d = nc.dram_tensor("wv", [CD, INNER], BF16, kind="ExternalInput").ap()
    wo_d = nc.dram_tensor("wo", [INNER, QD], BF16, kind="ExternalInput").ap()
    bo_d = nc.dram_tensor("bo", [QD], BF16, kind="ExternalInput").ap()
    out_d = nc.dram_tensor("out", [NQ, QD], F32, kind="ExternalOutput").ap()
    io = (xt, ctx_t, mask64, wq_d, wk_d, wv_d, wo_d, bo_d, out_d)
    with tile.TileContext(nc) as tc:
        if loop > 1:
            with tc.For_i(0, loop, 1, staggered_reset=staggered,
                          hint_engines=(mybir.EngineType.PE,)):
                for _ in range(iters):
                    _emit(tc, io, nkc, stages)
        else:
            for _ in range(iters):
                _emit(tc, io, nkc, stages)
    nc.compile()
    _CACHED[key] = nc
    return nc


def make_in_maps(x, context, mask, Wq, Wk, Wv, Wo, bo):
    x = np.asarray(x, dtype=np.float32)
    context = np.asarray(context, dtype=np.float32)
    mask_b = np.asarray(mask).astype(bool)
    Wq = np.ascontiguousarray(np.asarray(Wq, dtype=np.float32)).astype(NPBF16)
    Wk = np.ascontiguousarray(np.asarray(Wk, dtype=np.float32)).astype(NPBF16)
    Wv = np.ascontiguousarray(np.asarray(Wv, dtype=np.float32)).astype(NPBF16)
    Wo = np.ascontiguousarray(np.asarray(Wo, dtype=np.float32)).astype(NPBF16)
    bo = np.ascontiguousarray(np.asarray(bo, dtype=np.float32)).astype(NPBF16)

    counts = mask_b.sum(axis=1)
    n_max = max(int(counts.max()), 1)
    nkc = (n_max + P - 1) // P
    NKVP = nkc * P

    # a fully-masked batch reduces to uniform attention over all keys:
    # emulate exactly by sending the full context unmasked with Wq zeroed
    # (s = 0 -> softmax uniform), matching the reference's -inf softmax
    if (counts == 0).any():
        nkc = NKV // P
        NKVP = nkc * P

    ctx_ts, m64s, wq_zero = [], [], []
    for b in range(4):
        idx = np.nonzero(mask_b[b])[0]
        n = len(idx)
        wq_zero.append(n == 0)
        ctx_c = np.zeros((NKVP, CD), np.float32)
        if n:
            ctx_c[:n] = context[b][idx]
        else:
            n = NKV
            ctx_c[:n] = context[b]
        ctx_ts.append(np.ascontiguousarray(ctx_c.T).astype(NPBF16))
        valid = (np.arange(NKVP) < n).reshape(nkc, P)      # [c, p]
        m64 = np.repeat(valid.T[:, :, None], D, axis=2)    # [p, c, 64]
        m64s.append(np.ascontiguousarray(
            m64.reshape(P, nkc * D)).astype(NPBF16))

    in_maps = []
    for b in range(4):
        for qh in range(2):
            in_maps.append({
                "xt": np.ascontiguousarray(
                    x[b, qh * NQ:(qh + 1) * NQ, :].T).astype(NPBF16),
                "ctx_t": ctx_ts[b],
                "mask64": m64s[b],
                "wq": np.zeros_like(Wq) if wq_zero[b] else Wq,
                "wk": Wk, "wv": Wv, "wo": Wo, "bo": bo,
            })
    return in_maps, nkc


def run_sharded(x, context, mask, Wq, Wk, Wv, Wo, bo, trace=False, **kw):
    in_maps, nkc = make_in_maps(x, context, mask, Wq, Wk, Wv, Wo, bo)
    stages = ("proj", "attn", "out") + (
        () if np.asarray(bo).any() else ("nobias",))
    nc = _build(nkc=nkc, stages=stages)
    res = run_bass_kernel_spmd(nc, in_maps, list(range(8)), trace=trace, **kw)
    out = np.empty((4, 2 * NQ, QD), dtype=np.float32)
    for i in range(8):
        b, qh = divmod(i, 2)
        out[b, qh * NQ:(qh + 1) * NQ, :] = res.results[i]["out"]
    return out, res


def kernel(x, context, mask, Wq, Wk, Wv, Wo, bo):
    out, _ = run_sharded(x, context, mask, Wq, Wk, Wv, Wo, bo, trace=False)
    return out



# revision 38
# speedup vs baseline: 1.2092x; 1.0255x over previous
"""Trainium2 Bass kernel for cross-attention (b=4, nq=2048, nkv=1024,
qdim=1024, cdim=768, heads=16, dim_head=64).

Sharding: 8 cores = batch(4) x nq-half(2). Each core computes a disjoint
[1024, 1024] slice of the output; no collectives needed.

Host-side prep (part of kernel()):
  - KV compaction: the per-key mask drops ~half the keys. Gather the
    unmasked context rows per batch, pad with zeros to a multiple of 128
    (NKVP = nkc*128 kv positions). softmax over masked entries ==
    softmax over the unmasked subset, so this is exact.
  - x and ctx are uploaded pre-transposed (xt = x^T, ctx_t = ctx^T), which
    removes all PE transpose work on device.
  - mask64 ([128, nkc*64], bf16): per-kv-chunk validity column-broadcast
    to 64 cols; used as matmul weights for the softmax rowsum so padding
    rows are excluded. Pad rows of ctx are zero so V pad rows are zero.

Per-core device algorithm:
  KT = Wk^T @ CT   [inner, NKVP]   (inner chunk p holds heads 2p, 2p+1)
  V  = CT^T @ Wv   [NKVP, inner]   (pad rows zero by construction)
  QT = Wq^T @ XT   [inner, nq]     (single pass, all 8 Wq chunks resident)
  per head-pair p, per q-half hf, per kv-chunk c:
    S^T = K_h @ Q_h^T              (row-tiled K=64: 2 heads concurrent)
    ES  = exp(SCALE * S^T)         (ScalarE, PSUM->SBUF, bf16)
    OT_acc  += V_h^T @ ES          (col-tiled M=64: 2 heads concurrent)
    rs_acc  += mask64^T @ ES       (col-tiled M=64, excludes pad rows)
  rT = 1/rs ; OT = OT_acc * rT (bf16)
  out = (OT^T stacked) @ Wo + 1 x bo  (bf16 matmuls, bias via K=1 matmul)

Scores are O(1) by construction so unshifted exp is safe.
"""

import numpy as np
from contextlib import ExitStack

import ml_dtypes

import concourse.bass as bass
import concourse.mybir as mybir
import concourse.tile as tile
from concourse import bacc
from concourse.bass_utils import run_bass_kernel_spmd

F32 = mybir.dt.float32
F32R = mybir.dt.float32r
BF16 = mybir.dt.bfloat16
AF = mybir.ActivationFunctionType
NPBF16 = ml_dtypes.bfloat16

NQ = 1024      # queries per core
NKV = 1024
QD = 1024
CD = 768
H = 16
D = 64
INNER = 1024
SCALE = D ** -0.5
P = 128
NQC = NQ // P      # 8 nq chunks
QDC = QD // P      # 8
CDC = CD // P      # 6
HP = H // 2        # 8 head pairs


def R(ap):
    return ap.bitcast(F32R)


def _emit(tc, io, nkc, stages=("proj", "attn", "out")):
    nc = tc.nc
    xt_d, ctx_t, mask64_d, wq_d, wk_d, wv_d, wo_d, bo_d, out_d = io
    NKVP = nkc * P
    do_proj = "proj" in stages
    attn_lv = (4 if "attn" in stages else 3 if "attn3" in stages
               else 2 if "attn2" in stages else 1 if "attn1" in stages else 0)
    do_out = "out" in stages
    has_bias = "nobias" not in stages

    with ExitStack() as top:
        # ---- flat pools: everything resident, no pool-boundary barriers ----
        const = top.enter_context(tc.tile_pool(name="const", bufs=1))
        big = top.enter_context(tc.tile_pool(name="big", bufs=1))

        ones = const.tile([1, P], BF16, tag="ones")
        nc.vector.memset(ones[:], 1.0)
        mask64 = const.tile([P, nkc * D], BF16, tag="mask64")
        nc.sync.dma_start(out=mask64[:], in_=mask64_d)

        ot = big.tile([P, QDC * NQ], BF16, tag="ot")     # O^T: chunk k cols k*NQ..
        qt = big.tile([P, HP * NQ], BF16, tag="qt")      # Q^T: chunk p cols p*NQ..
        kt = big.tile([P, HP * NKVP], BF16, tag="kt")
        vt = big.tile([P, nkc * INNER], BF16, tag="vt")  # V: chunk c cols c*INNER..

        # ---- input loads (few big DMAs, split across both HWDGE queues;
        # ordered so K-proj deps (ct on sync, wk on scalar) land first) ----
        ct = big.tile([P, CDC * NKVP], BF16, tag="ct")
        ct3 = ct.rearrange("p (k n) -> p k n", n=NKVP)
        nc.sync.dma_start(out=ct3[:], in_=ctx_t.rearrange("(k p) n -> p k n", p=P))
        wkt = big.tile([P, CDC * INNER], BF16, tag="wk")
        wk3 = wkt.rearrange("p (k n) -> p k n", n=INNER)
        nc.scalar.dma_start(out=wk3[:], in_=wk_d.rearrange("(k p) n -> p k n", p=P))
        wk = [wk3[:, k, :] for k in range(CDC)]
        xt = big.tile([P, QDC * NQ], BF16, tag="xt")
        xt3 = xt.rearrange("p (k n) -> p k n", n=NQ)
        nc.sync.dma_start(out=xt3[:], in_=xt_d.rearrange("(k p) n -> p k n", p=P))
        wqt = big.tile([P, QDC * INNER], BF16, tag="wq")
        wq3 = wqt.rearrange("p (k n) -> p k n", n=INNER)
        nc.scalar.dma_start(out=wq3[:], in_=wq_d.rearrange("(k p) n -> p k n", p=P))
        wq = [wq3[:, k, :] for k in range(QDC)]
        wvt = big.tile([P, CDC * INNER], BF16, tag="wv")
        wv3 = wvt.rearrange("p (k n) -> p k n", n=INNER)
        nc.sync.dma_start(out=wv3[:], in_=wv_d.rearrange("(k p) n -> p k n", p=P))
        wv = [wv3[:, k, :] for k in range(CDC)]
        wo = big.tile([P, QDC * QD], BF16, tag="wo")
        wo3 = wo.rearrange("p (k n) -> p k n", n=QD)
        nc.scalar.dma_start(out=wo3[:], in_=wo_d.rearrange("(k p) n -> p k n", p=P))
        bo_t = const.tile([1, QD], BF16, tag="bo")
        nc.sync.dma_start(out=bo_t[:], in_=bo_d[:].rearrange("(o n) -> o n", o=1))

        # ---- K projection: KT[inner, kv] (psum must not cross 2KB banks) ----
        ksplits = [(lo, min(lo + 512, NKVP)) for lo in range(0, NKVP, 512)]
        proj_ctx = ExitStack()
        pj_ps = proj_ctx.enter_context(tc.tile_pool(name="pj_ps", bufs=2, space="PSUM"))
        for p in range(HP if do_proj else 0):
            ps = pj_ps.tile([P, NKVP], F32, tag="pj", name=f"pjk{p}")
            for k in range(CDC):
                for (lo, hi) in ksplits:
                    nc.tensor.matmul(
                        ps[:, lo:hi], wk[k][:, p * P:(p + 1) * P],
                        ct[:, k * NKVP + lo: k * NKVP + hi],
                        start=(k == 0), stop=(k == CDC - 1),
                        skip_group_check=True)
            nc.vector.tensor_copy(kt[:, p * NKVP:(p + 1) * NKVP], ps[:])

        # ---- V projection ----
        vq_ps = proj_ctx.enter_context(tc.tile_pool(name="vq_ps", bufs=4, space="PSUM"))
        for c in range(nkc if do_proj else 0):
            for hf in range(2):
                ps = vq_ps.tile([P, 512], F32, tag="vq", name=f"pjv{c}_{hf}")
                for k in range(CDC):
                    nc.tensor.matmul(
                        ps[:], ct[:, k * NKVP + c * P: k * NKVP + (c + 1) * P],
                        wv[k][:, hf * 512:(hf + 1) * 512],
                        start=(k == 0), stop=(k == CDC - 1))
                nc.vector.tensor_copy(
                    vt[:, c * INNER + hf * 512: c * INNER + (hf + 1) * 512],
                    ps[:])

        # ---- Q projection ----
        for p in range(HP if do_proj else 0):
            for hf in range(2):
                ps = vq_ps.tile([P, 512], F32, tag="vq", name=f"pjq{p}_{hf}")
                for k in range(QDC):
                    nc.tensor.matmul(
                        ps[:], wq[k][:, p * P:(p + 1) * P],
                        xt[:, k * NQ + hf * 512: k * NQ + (hf + 1) * 512],
                        start=(k == 0), stop=(k == QDC - 1))
                nc.vector.tensor_copy(
                    qt[:, p * NQ + hf * 512: p * NQ + (hf + 1) * 512], ps[:])

        # ---- attention ----
        proj_ctx.close()
        attn_ctx = ExitStack()
        es_pool = attn_ctx.enter_context(tc.tile_pool(name="esp", bufs=5))
        rt_pool = attn_ctx.enter_context(tc.tile_pool(name="rtp", bufs=2))
        ps_s = attn_ctx.enter_context(tc.tile_pool(name="ps_s", bufs=3, space="PSUM"))
        ps_o = attn_ctx.enter_context(tc.tile_pool(name="ps_o", bufs=1, space="PSUM"))
        ps_r = attn_ctx.enter_context(tc.tile_pool(name="ps_r", bufs=1, space="PSUM"))
        for p in range(HP if attn_lv else 0):
            for hf in range(2):
                po = ps_o.tile([P, 512], F32, tag="po", name=f"po{p}_{hf}")
                pr = ps_r.tile([P, 512], F32, tag="pr", name=f"pr{p}_{hf}")
                for c in range(nkc):
                    # S^T for both heads of the pair (row-tiled K=64):
                    # head h -> cols 0:512, head h\' -> cols 512:1024
                    ps = ps_s.tile([P, NQ], F32, tag="ss", name=f"ss{p}_{hf}_{c}")
                    for hh in range(2):
                        nc.tensor.matmul(
                            ps[:, hh * 512:(hh + 1) * 512],
                            kt[hh * D:(hh + 1) * D,
                               p * NKVP + c * P: p * NKVP + (c + 1) * P],
                            qt[hh * D:(hh + 1) * D,
                               p * NQ + hf * 512: p * NQ + (hf + 1) * 512],
                            start=True, stop=True,
                            tile_position=(hh * D, 0))
                    if attn_lv < 2:
                        continue
                    es = es_pool.tile([P, NQ], BF16, tag="es",
                                      name=f"es{p}_{hf}_{c}")
                    nc.scalar.activation(es[:], ps[:], AF.Exp, scale=float(SCALE))
                    for hh in range(2 if attn_lv >= 3 else 0):
                        h = 2 * p + hh
                        esl = es[:, hh * 512:(hh + 1) * 512]
                        nc.tensor.matmul(
                            po[hh * D:(hh + 1) * D, :],
                            vt[:, c * INNER + h * D: c * INNER + (h + 1) * D],
                            esl,
                            start=(c == 0), stop=(c == nkc - 1),
                            tile_position=(0, hh * D),
                            skip_group_check=True)
                        if attn_lv >= 4:
                            nc.tensor.matmul(
                                pr[hh * D:(hh + 1) * D, :],
                                mask64[:, c * D:(c + 1) * D], esl,
                                start=(c == 0), stop=(c == nkc - 1),
                                tile_position=(0, hh * D),
                                skip_group_check=True)
                if attn_lv < 4:
                    continue
                # epilogue: normalize this (pair, nq-half) slice
                rt = rt_pool.tile([P, 512], F32, tag="rt", name=f"rt{p}_{hf}")
                with nc.allow_low_precision(reason="softmax reciprocal"):
                    nc.vector.reciprocal(rt[:], pr[:])
                nc.vector.tensor_mul(
                    ot[:, p * NQ + hf * 512: p * NQ + (hf + 1) * 512],
                    po[:], rt[:])


        # ---- output projection ----
        attn_ctx.close()
        out_ps = top.enter_context(tc.tile_pool(name="out_ps", bufs=6, space="PSUM"))
        out_sb = top.enter_context(tc.tile_pool(name="out_sb", bufs=3))
        for m in range(NQC if do_out else 0):
            sb = out_sb.tile([P, QD], F32, tag="osb", name=f"osb{m}")
            for n in range(2):
                ps = out_ps.tile([P, 512], F32, tag="ops", name=f"ops{m}_{n}")
                for k in range(QDC):
                    nc.tensor.matmul(
                        ps[:],
                        ot[:, k * NQ + m * P: k * NQ + (m + 1) * P],
                        wo3[:, k, n * 512:(n + 1) * 512],
                        start=(k == 0),
                        stop=(k == QDC - 1) and not has_bias,
                        skip_group_check=True)
                if has_bias:
                    nc.tensor.matmul(
                        ps[:], ones[0:1, 0:P],
                        bo_t[0:1, n * 512:(n + 1) * 512],
                        start=False, stop=True, skip_group_check=True)
                nc.vector.tensor_copy(sb[:, n * 512:(n + 1) * 512], ps[:])
            nc.sync.dma_start(out=out_d[m * P:(m + 1) * P, :], in_=sb[:])

_CACHED = {}


def _build(iters=1, loop=1, nkc=5, stages=("proj", "attn", "out"), staggered=False):
    """Build the program. `iters` unrolls the body in the instruction stream;
    `loop` wraps it in an on-device hardware loop (constant program size) —
    used by test.py to measure per-body device time as a slope. `nkc` is the
    number of 128-row kv chunks after mask compaction. `stages` restricts the
    emitted phases (timing probes only — output is garbage unless full)."""
    key = (iters, loop, nkc, tuple(stages), staggered)
    if key in _CACHED:
        return _CACHED[key]
    NKVP = nkc * P
    nc = bacc.Bacc("TRN2", debug=False, target_bir_lowering=False)
    xt = nc.dram_tensor("xt", [QD, NQ], BF16, kind="ExternalInput").ap()
    ctx_t = nc.dram_tensor("ctx_t", [CD, NKVP], BF16, kind="ExternalInput").ap()
    mask64 = nc.dram_tensor("mask64", [P, nkc * D], BF16,
                            kind="ExternalInput").ap()
    wq_d = nc.dram_tensor("wq", [QD, INNER], BF16, kind="ExternalInput").ap()
    wk_d = nc.dram_tensor("wk", [CD, INNER], BF16, kind="ExternalInput").ap()
    wv_d = nc.dram_tensor("wv", [CD, INNER], BF16, kind="ExternalInput").ap()
    wo_d = nc.dram_tensor("wo", [INNER, QD], BF16, kind="ExternalInput").ap()
    bo_d = nc.dram_tensor("bo", [QD], BF16, kind="ExternalInput").ap()
    out_d = nc.dram_tensor("out", [NQ, QD], F32, kind="ExternalOutput").ap()
    io = (xt, ctx_t, mask64, wq_d, wk_d, wv_d, wo_d, bo_d, out_d)
    with tile.TileContext(nc) as tc:
        if loop > 1:
            with tc.For_i(0, loop, 1, staggered_reset=staggered,
                          hint_engines=(mybir.EngineType.PE,)):
                for _ in range(iters):
                    _emit(tc, io, nkc, stages)
        else:
            for _ in range(iters):
                _emit(tc, io, nkc, stages)
    nc.compile()
    _CACHED[key] = nc
    return nc


def make_in_maps(x, context, mask, Wq, Wk, Wv, Wo, bo):
    x = np.asarray(x, dtype=np.float32)
    context = np.asarray(context, dtype=np.float32)
    mask_b = np.asarray(mask).astype(bool)
    Wq = np.ascontiguousarray(np.asarray(Wq, dtype=np.float32)).astype(NPBF16)
    Wk = np.ascontiguousarray(np.asarray(Wk, dtype=np.float32)).astype(NPBF16)
    Wv = np.ascontiguousarray(np.asarray(Wv, dtype=np.float32)).astype(NPBF16)
    Wo = np.ascontiguousarray(np.asarray(Wo, dtype=np.float32)).astype(NPBF16)
    bo = np.ascontiguousarray(np.asarray(bo, dtype=np.float32)).astype(NPBF16)

    counts = mask_b.sum(axis=1)
    n_max = max(int(counts.max()), 1)
    nkc = (n_max + P - 1) // P
    NKVP = nkc * P

    # a fully-masked batch reduces to uniform attention over all keys:
    # emulate exactly by sending the full context unmasked with Wq zeroed
    # (s = 0 -> softmax uniform), matching the reference's -inf softmax
    if (counts == 0).any():
        nkc = NKV // P
        NKVP = nkc * P

    ctx_ts, m64s, wq_zero = [], [], []
    for b in range(4):
        idx = np.nonzero(mask_b[b])[0]
        n = len(idx)
        wq_zero.append(n == 0)
        ctx_c = np.zeros((NKVP, CD), np.float32)
        if n:
            ctx_c[:n] = context[b][idx]
        else:
            n = NKV
            ctx_c[:n] = context[b]
        ctx_ts.append(np.ascontiguousarray(ctx_c.T).astype(NPBF16))
        valid = (np.arange(NKVP) < n).reshape(nkc, P)      # [c, p]
        m64 = np.repeat(valid.T[:, :, None], D, axis=2)    # [p, c, 64]
        m64s.append(np.ascontiguousarray(
            m64.reshape(P, nkc * D)).astype(NPBF16))

    in_maps = []
    for b in range(4):
        for qh in range(2):
            in_maps.append({
                "xt": np.ascontiguousarray(
                    x[b, qh * NQ:(qh + 1) * NQ, :].T).astype(NPBF16),
                "ctx_t": ctx_ts[b],
                "mask64": m64s[b],
                "wq": np.zeros_like(Wq) if wq_zero[b] else Wq,
                "wk": Wk, "wv": Wv, "wo": Wo, "bo": bo,
            })
    return in_maps, nkc


def run_sharded(x, context, mask, Wq, Wk, Wv, Wo, bo, trace=False, **kw):
    in_maps, nkc = make_in_maps(x, context, mask, Wq, Wk, Wv, Wo, bo)
    stages = ("proj", "attn", "out") + (
        () if np.asarray(bo).any() else ("nobias",))
    nc = _build(nkc=nkc, stages=stages)
    res = run_bass_kernel_spmd(nc, in_maps, list(range(8)), trace=trace, **kw)
    out = np.empty((4, 2 * NQ, QD), dtype=np.float32)
    for i in range(8):
        b, qh = divmod(i, 2)
        out[b, qh * NQ:(qh + 1) * NQ, :] = res.results[i]["out"]
    return out, res


def kernel(x, context, mask, Wq, Wk, Wv, Wo, bo):
    out, _ = run_sharded(x, context, mask, Wq, Wk, Wv, Wo, bo, trace=False)
    return out

